# revision 6
# baseline (speedup 1.0000x reference)
"""Trainium2 Bass kernel for nn_Gate_Net (Toeplitz + hard-sigmoid prob + cumprod).

Per (doc-dir, column-block) tile, [128 cols x 1024 rows]:
  Pool:     Q[:, W:] = 0 + chat_p                  (tail constant fill, fp32)
  ACT/Pool: Q[:, :W] = relu(B[:, y:y+W] + c_p)     (fp16 B + fp32 bias -> fp32)
  Pool:     Q[:, :W] = min(Q[:, :W], 1)            (clamp above)
  DVE:      R = cumprod scan along rows            (fp32 state -> bf16 out;
                                                    scans are DVE-only on HW)
  PE:       8 transposes R -> one psum tile [128, 1024] bf16
  ACT/DVE:  one copy psum -> outsb bf16            (GPSIMD cannot read PSUM)
  DMA:      per dd, 8 stores outsb -> out (bf16 DRAM)

bf16 output halves HBM writes + host transfer; fp16 B halves input DMA
(verified rel-err 1.6e-3 vs the 2e-2 gate). Scan state is fp32 internally so
tail factors are not compounded in bf16. Engine schedules are load-balanced.
"""
import numpy as np

import concourse.bass as bass
import concourse.bacc as bacc
import concourse.tile as tile
from concourse import mybir
from concourse import bass_utils

P = 128
N = 1022          # columns j per doc-dir
ROWS = N - 1      # 1021 output rows
NB = 8            # column blocks (last has 126 valid columns)
MB = 8            # row blocks (last has 125 valid rows)
ARRW = 1152       # padded diag-source array width
BW = 1024         # sheared tile width
FD = 1024         # padded scan length (3 pad rows -> garbage partitions)
OUTW = MB * N     # 8176 free-dim of the per-dd output staging tile

_NC_CACHE: dict = {}

f32 = mybir.dt.float32
f16 = mybir.dt.float16
bf16 = mybir.dt.bfloat16


def _weighted_schedule(weights, n):
    """Static engine schedule of length n with given integer weights."""
    total = sum(weights)
    sched = []
    acc = [0.0] * len(weights)
    for i in range(n):
        for k in range(len(weights)):
            acc[k] += weights[k] / total
        k = max(range(len(weights)), key=lambda j: acc[j])
        acc[k] -= 1.0
        sched.append(k)
    return sched


def build_nc(n_dd: int = 8, main_w=(34, 30), copy_w=(53, 11)):
    """main_w: (ACT, Pool); copy_w: (ACT, DVE)."""
    nc = bacc.Bacc("TRN2", target_bir_lowering=False, debug=False, num_devices=8)
    arr = nc.dram_tensor("arr", [n_dd, ARRW], f16, kind="ExternalInput")
    cc = nc.dram_tensor("cc", [n_dd, P, 16], f32, kind="ExternalInput")
    out = nc.dram_tensor("out", [n_dd, ROWS, N], bf16, kind="ExternalOutput")

    add_op = mybir.AluOpType.add
    min_op = mybir.AluOpType.min
    mult_op = mybir.AluOpType.mult
    relu = mybir.ActivationFunctionType.Relu

    n_tiles = n_dd * NB
    main_sched = _weighted_schedule(main_w, n_tiles)   # 0=ACT 1=Pool
    copy_sched = _weighted_schedule(copy_w, n_tiles)   # 0=ACT 1=DVE

    with tile.TileContext(nc) as tc:
        with (
            tc.tile_pool(name="consts", bufs=1) as consts,
            tc.tile_pool(name="bsrc", bufs=2) as bsrc_pool,
            tc.tile_pool(name="qpool", bufs=6) as qpool,
            tc.tile_pool(name="rpool", bufs=6) as rpool,
            tc.tile_pool(name="outp", bufs=3) as outp,
            tc.tile_pool(name="psum", bufs=4, space="PSUM") as psum,
        ):
            # flip permutation: flip[k, b] = 1 iff k + b == P-1, so the PE
            # transpose emits columns in natural j order (partition p of the
            # scan tiles holds column j = jb*128 + 127 - p).
            flipf = consts.tile([P, P], f32)
            nc.gpsimd.memset(flipf[:], 0.0)
            nc.gpsimd.affine_select(
                out=flipf[:], in_=flipf[:],
                compare_op=mybir.AluOpType.not_equal, fill=1.0,
                base=-(P - 1), pattern=[[1, P]], channel_multiplier=1,
            )
            flip = consts.tile([P, P], bf16)
            nc.vector.tensor_copy(flip[:], flipf[:])
            zeros = consts.tile([P, FD], f32)
            nc.vector.memset(zeros[:], 0.0)
            # warm the ACT spline table before any data arrives
            warm = consts.tile([P, 1], f32)
            nc.scalar.activation(out=warm[:], in_=zeros[:, 0:1], func=relu,
                                 bias=0.0, scale=1.0)

            # all doc-dirs' column constants in one DMA: csb[:, dd*16 + k]
            csb = consts.tile([P, n_dd * 16], f32)
            cc_src = bass.AP(
                tensor=cc, offset=0, ap=[[16, P], [P * 16, n_dd], [1, 16]]
            )
            csb3 = csb.rearrange("p (d i) -> p d i", d=n_dd)
            nc.sync.dma_start(out=csb3[:, :, :], in_=cc_src)

            tile_idx = 0
            for dd in range(n_dd):
                B = bsrc_pool.tile([P, BW], f16, tag="B", name="B")
                diag_src = bass.AP(
                    tensor=arr, offset=dd * ARRW, ap=[[1, P], [1, BW]]
                )
                nc.sync.dma_start(out=B[:], in_=diag_src)

                outsb = outp.tile([P, OUTW], bf16, tag="outsb", name="outsb")
                outsb3 = outsb.rearrange("p (m j) -> p m j", m=MB)

                for jb in range(NB):
                    W = min(jb * 128 + 128, ROWS)
                    y = 896 - jb * 128
                    cols = 126 if jb == NB - 1 else 128
                    cbias = csb[:, dd * 16 + jb:dd * 16 + jb + 1]
                    chat = csb[:, dd * 16 + 8 + jb:dd * 16 + 8 + jb + 1]
                    Q = qpool.tile([P, FD], f32, tag="Q", name="Q")
                    # tail first: independent of main/min (Pool)
                    nc.gpsimd.tensor_scalar(
                        out=Q[:, W:FD], in0=zeros[:, 0:FD - W],
                        scalar1=chat, scalar2=None, op0=add_op,
                    )
                    # main: relu(B + c_j) = (B + c) max 0, fp32
                    if main_sched[tile_idx] == 0:
                        nc.scalar.activation(
                            out=Q[:, 0:W], in_=B[:, y:y + W], func=relu,
                            bias=cbias, scale=1.0,
                        )
                    else:
                        nc.gpsimd.tensor_scalar(
                            out=Q[:, 0:W], in0=B[:, y:y + W],
                            scalar1=cbias, scalar2=0.0,
                            op0=add_op, op1=mybir.AluOpType.max,
                        )
                    # clamp above: min(Q, 1) in place (Pool)
                    nc.gpsimd.tensor_scalar(
                        out=Q[:, 0:W], in0=Q[:, 0:W],
                        scalar1=1.0, scalar2=None, op0=min_op,
                    )
                    R = rpool.tile([P, FD], bf16, tag="R", name="R")
                    nc.vector.tensor_tensor_scan(
                        out=R[:], data0=Q[:], data1=zeros[:],
                        initial=1.0, op0=mult_op, op1=add_op,
                    )
                    # 8 PE transposes into one psum tile: pt[:, mb*128 + b]
                    # holds output row mb*128+p, column jb*128+b.
                    pt = psum.tile([P, 1024], bf16, tag="pt", name="pt")
                    for mb in range(MB):
                        nc.tensor.transpose(
                            pt[:, mb * 128:mb * 128 + 128],
                            R[:, mb * 128:mb * 128 + 128],
                            flip[:],
                        )
                    # one egress copy PSUM -> outsb (GPSIMD cannot read PSUM)
                    pt3 = pt.rearrange("p (m b) -> p m b", m=MB)
                    if copy_sched[tile_idx] == 0:
                        nc.scalar.copy(
                            out=outsb3[:, :, jb * 128:jb * 128 + cols],
                            in_=pt3[:, :, 0:cols],
                        )
                    else:
                        nc.vector.tensor_copy(
                            outsb3[:, :, jb * 128:jb * 128 + cols],
                            pt3[:, :, 0:cols],
                        )
                    tile_idx += 1

                for mb in range(MB):
                    chunk = 125 if mb == MB - 1 else 128
                    # last doc-dir: drain through both HWDGE queues (ACT is
                    # idle during the drain, so charging its queue is free)
                    deng = nc.scalar if (dd == n_dd - 1 and mb % 2 == 1) else nc.sync
                    deng.dma_start(
                        out=out[dd, mb * 128:mb * 128 + chunk, :],
                        in_=outsb[:chunk, mb * N:mb * N + N],
                    )
    nc.compile()
    return nc


def get_nc(n_dd: int = 8):
    if n_dd not in _NC_CACHE:
        _NC_CACHE[n_dd] = build_nc(n_dd)
    return _NC_CACHE[n_dd]


def make_core_inputs(docs_core: np.ndarray) -> dict:
    """docs_core: [n_docs, 1024] f32 -> in_map with arr/cc for n_docs*2 doc-dirs."""
    n_docs = docs_core.shape[0]
    n_dd = n_docs * 2
    arr = np.zeros((n_dd, ARRW), np.float16)
    cc = np.zeros((n_dd, P, 16), np.float32)
    for dl in range(n_docs):
        s = docs_core[dl, 1:-1].astype(np.float32)  # 1022
        for t in range(2):
            v = s if t == 0 else s[::-1]
            dd = dl * 2 + t
            v10 = (np.float32(10.0) * v).astype(np.float32)
            arr[dd, 1:1 + N] = v10[::-1].astype(np.float16)
            cvals = (np.float32(1.0) - v10).astype(np.float32)
            # partition p holds column j = jb*128 + (127 - p)
            for jb in range(NB):
                seg = cvals[jb * 128: jb * 128 + 128]
                cseg = np.zeros(P, np.float32)
                cseg[P - len(seg):] = seg[::-1]
                cc[dd, :, jb] = cseg
                cc[dd, :, 8 + jb] = np.minimum(cseg, np.float32(1.0))
    return {"arr": arr, "cc": cc}


def kernel(score: np.ndarray, score_idx: np.ndarray) -> np.ndarray:
    score = np.asarray(score, dtype=np.float32)
    score_idx = np.asarray(score_idx)
    docs = score[score_idx]  # [B, L] gather
    Bn, L = docs.shape       # 32, 1024
    n_cores = 8
    docs_per_core = Bn // n_cores  # 4

    in_maps = [
        make_core_inputs(docs[c * docs_per_core:(c + 1) * docs_per_core])
        for c in range(n_cores)
    ]
    nc = get_nc(docs_per_core * 2)
    res = bass_utils.run_bass_kernel_spmd(nc, in_maps, core_ids=list(range(n_cores)))
    full = np.empty((Bn, 2, ROWS, N), np.float32)
    for c in range(n_cores):
        o = np.asarray(res.results[c]["out"]).reshape(docs_per_core, 2, ROWS, N)
        full[c * docs_per_core:(c + 1) * docs_per_core] = o.astype(np.float32)
    return full


# revision 7
# speedup vs baseline: 1.0111x; 1.0111x over previous
"""Trainium2 Bass kernel for nn_Gate_Net (Toeplitz + hard-sigmoid prob + cumprod).

Per (doc-dir, column-block) tile, [128 cols x 1024 rows]:
  Pool:     Q[:, W:] = 0 + chat_p                  (tail constant fill, fp32)
  ACT/Pool: Q[:, :W] = relu(B[:, y:y+W] + c_p)     (fp16 B + fp32 bias -> fp32)
  Pool:     Q[:, :W] = min(Q[:, :W], 1)            (clamp above)
  DVE:      R = cumprod scan along rows            (fp32 state -> bf16 out;
                                                    scans are DVE-only on HW)
  PE:       8 transposes R -> one psum tile [128, 1024] bf16
  ACT/DVE:  one copy psum -> outsb bf16            (GPSIMD cannot read PSUM)
  DMA:      per dd, 8 stores outsb -> out (bf16 DRAM)

bf16 output halves HBM writes + host transfer; fp16 B halves input DMA
(verified rel-err 1.6e-3 vs the 2e-2 gate). Scan state is fp32 internally so
tail factors are not compounded in bf16. Engine schedules are load-balanced.
"""
import numpy as np

import concourse.bass as bass
import concourse.bacc as bacc
import concourse.tile as tile
from concourse import mybir
from concourse import bass_utils

P = 128
N = 1022          # columns j per doc-dir
ROWS = N - 1      # 1021 output rows
NB = 8            # column blocks (last has 126 valid columns)
MB = 8            # row blocks (last has 125 valid rows)
ARRW = 1152       # padded diag-source array width
BW = 1024         # sheared tile width
FD = 1024         # padded scan length (3 pad rows -> garbage partitions)
OUTW = MB * N     # 8176 free-dim of the per-dd output staging tile

_NC_CACHE: dict = {}

f32 = mybir.dt.float32
f16 = mybir.dt.float16
bf16 = mybir.dt.bfloat16


def _weighted_schedule(weights, n):
    """Static engine schedule of length n with given integer weights."""
    total = sum(weights)
    sched = []
    acc = [0.0] * len(weights)
    for i in range(n):
        for k in range(len(weights)):
            acc[k] += weights[k] / total
        k = max(range(len(weights)), key=lambda j: acc[j])
        acc[k] -= 1.0
        sched.append(k)
    return sched


def build_nc(n_dd: int = 8, main_w=(29, 35), copy_w=(55, 9)):
    """main_w: (ACT, Pool); copy_w: (ACT, DVE)."""
    nc = bacc.Bacc("TRN2", target_bir_lowering=False, debug=False, num_devices=8)
    arr = nc.dram_tensor("arr", [n_dd, ARRW], f16, kind="ExternalInput")
    cc = nc.dram_tensor("cc", [n_dd, P, 16], f32, kind="ExternalInput")
    out = nc.dram_tensor("out", [n_dd, ROWS, N], bf16, kind="ExternalOutput")

    add_op = mybir.AluOpType.add
    min_op = mybir.AluOpType.min
    mult_op = mybir.AluOpType.mult
    relu = mybir.ActivationFunctionType.Relu

    n_tiles = n_dd * NB
    main_sched = _weighted_schedule(main_w, n_tiles)   # 0=ACT 1=Pool
    copy_sched = _weighted_schedule(copy_w, n_tiles)   # 0=ACT 1=DVE

    with tile.TileContext(nc) as tc:
        with (
            tc.tile_pool(name="consts", bufs=1) as consts,
            tc.tile_pool(name="bsrc", bufs=2) as bsrc_pool,
            tc.tile_pool(name="qpool", bufs=6) as qpool,
            tc.tile_pool(name="rpool", bufs=6) as rpool,
            tc.tile_pool(name="outp", bufs=3) as outp,
            tc.tile_pool(name="psum", bufs=4, space="PSUM") as psum,
        ):
            # flip permutation: flip[k, b] = 1 iff k + b == P-1, so the PE
            # transpose emits columns in natural j order (partition p of the
            # scan tiles holds column j = jb*128 + 127 - p).
            flipf = consts.tile([P, P], f32)
            nc.gpsimd.memset(flipf[:], 0.0)
            nc.gpsimd.affine_select(
                out=flipf[:], in_=flipf[:],
                compare_op=mybir.AluOpType.not_equal, fill=1.0,
                base=-(P - 1), pattern=[[1, P]], channel_multiplier=1,
            )
            flip = consts.tile([P, P], bf16)
            nc.vector.tensor_copy(flip[:], flipf[:])
            zeros = consts.tile([P, FD], f32)
            nc.vector.memset(zeros[:], 0.0)
            # warm the ACT spline table before any data arrives
            warm = consts.tile([P, 1], f32)
            nc.scalar.activation(out=warm[:], in_=zeros[:, 0:1], func=relu,
                                 bias=0.0, scale=1.0)

            # all doc-dirs' column constants in one DMA: csb[:, dd*16 + k]
            csb = consts.tile([P, n_dd * 16], f32)
            cc_src = bass.AP(
                tensor=cc, offset=0, ap=[[16, P], [P * 16, n_dd], [1, 16]]
            )
            csb3 = csb.rearrange("p (d i) -> p d i", d=n_dd)
            nc.sync.dma_start(out=csb3[:, :, :], in_=cc_src)

            tile_idx = 0
            for dd in range(n_dd):
                B = bsrc_pool.tile([P, BW], f16, tag="B", name="B")
                diag_src = bass.AP(
                    tensor=arr, offset=dd * ARRW, ap=[[1, P], [1, BW]]
                )
                nc.sync.dma_start(out=B[:], in_=diag_src)

                outsb = outp.tile([P, OUTW], bf16, tag="outsb", name="outsb")
                outsb3 = outsb.rearrange("p (m j) -> p m j", m=MB)

                for jb in range(NB):
                    W = min(jb * 128 + 128, ROWS)
                    y = 896 - jb * 128
                    cols = 126 if jb == NB - 1 else 128
                    cbias = csb[:, dd * 16 + jb:dd * 16 + jb + 1]
                    chat = csb[:, dd * 16 + 8 + jb:dd * 16 + 8 + jb + 1]
                    Q = qpool.tile([P, FD], f32, tag="Q", name="Q")
                    # tail first: independent of main/min (Pool)
                    nc.gpsimd.tensor_scalar(
                        out=Q[:, W:FD], in0=zeros[:, 0:FD - W],
                        scalar1=chat, scalar2=None, op0=add_op,
                    )
                    # main: relu(B + c_j) = (B + c) max 0, fp32
                    if main_sched[tile_idx] == 0:
                        nc.scalar.activation(
                            out=Q[:, 0:W], in_=B[:, y:y + W], func=relu,
                            bias=cbias, scale=1.0,
                        )
                    else:
                        nc.gpsimd.tensor_scalar(
                            out=Q[:, 0:W], in0=B[:, y:y + W],
                            scalar1=cbias, scalar2=0.0,
                            op0=add_op, op1=mybir.AluOpType.max,
                        )
                    # clamp above: min(Q, 1) in place (Pool)
                    nc.gpsimd.tensor_scalar(
                        out=Q[:, 0:W], in0=Q[:, 0:W],
                        scalar1=1.0, scalar2=None, op0=min_op,
                    )
                    R = rpool.tile([P, FD], bf16, tag="R", name="R")
                    nc.vector.tensor_tensor_scan(
                        out=R[:], data0=Q[:], data1=zeros[:],
                        initial=1.0, op0=mult_op, op1=add_op,
                    )
                    # 8 PE transposes into one psum tile: pt[:, mb*128 + b]
                    # holds output row mb*128+p, column jb*128+b.
                    pt = psum.tile([P, 1024], bf16, tag="pt", name="pt")
                    for mb in range(MB):
                        nc.tensor.transpose(
                            pt[:, mb * 128:mb * 128 + 128],
                            R[:, mb * 128:mb * 128 + 128],
                            flip[:],
                        )
                    # one egress copy PSUM -> outsb (GPSIMD cannot read PSUM)
                    pt3 = pt.rearrange("p (m b) -> p m b", m=MB)
                    if copy_sched[tile_idx] == 0:
                        nc.scalar.copy(
                            out=outsb3[:, :, jb * 128:jb * 128 + cols],
                            in_=pt3[:, :, 0:cols],
                        )
                    else:
                        nc.vector.tensor_copy(
                            outsb3[:, :, jb * 128:jb * 128 + cols],
                            pt3[:, :, 0:cols],
                        )
                    tile_idx += 1

                for mb in range(MB):
                    chunk = 125 if mb == MB - 1 else 128
                    # last doc-dir: drain through both HWDGE queues (ACT is
                    # idle during the drain, so charging its queue is free)
                    deng = nc.scalar if (dd == n_dd - 1 and mb % 2 == 1) else nc.sync
                    deng.dma_start(
                        out=out[dd, mb * 128:mb * 128 + chunk, :],
                        in_=outsb[:chunk, mb * N:mb * N + N],
                    )
    nc.compile()
    return nc


def get_nc(n_dd: int = 8):
    if n_dd not in _NC_CACHE:
        _NC_CACHE[n_dd] = build_nc(n_dd)
    return _NC_CACHE[n_dd]


def make_core_inputs(docs_core: np.ndarray) -> dict:
    """docs_core: [n_docs, 1024] f32 -> in_map with arr/cc for n_docs*2 doc-dirs."""
    n_docs = docs_core.shape[0]
    n_dd = n_docs * 2
    arr = np.zeros((n_dd, ARRW), np.float16)
    cc = np.zeros((n_dd, P, 16), np.float32)
    for dl in range(n_docs):
        s = docs_core[dl, 1:-1].astype(np.float32)  # 1022
        for t in range(2):
            v = s if t == 0 else s[::-1]
            dd = dl * 2 + t
            v10 = (np.float32(10.0) * v).astype(np.float32)
            arr[dd, 1:1 + N] = v10[::-1].astype(np.float16)
            cvals = (np.float32(1.0) - v10).astype(np.float32)
            # partition p holds column j = jb*128 + (127 - p)
            for jb in range(NB):
                seg = cvals[jb * 128: jb * 128 + 128]
                cseg = np.zeros(P, np.float32)
                cseg[P - len(seg):] = seg[::-1]
                cc[dd, :, jb] = cseg
                cc[dd, :, 8 + jb] = np.minimum(cseg, np.float32(1.0))
    return {"arr": arr, "cc": cc}


def kernel(score: np.ndarray, score_idx: np.ndarray) -> np.ndarray:
    score = np.asarray(score, dtype=np.float32)
    score_idx = np.asarray(score_idx)
    docs = score[score_idx]  # [B, L] gather
    Bn, L = docs.shape       # 32, 1024
    n_cores = 8
    docs_per_core = Bn // n_cores  # 4

    in_maps = [
        make_core_inputs(docs[c * docs_per_core:(c + 1) * docs_per_core])
        for c in range(n_cores)
    ]
    nc = get_nc(docs_per_core * 2)
    res = bass_utils.run_bass_kernel_spmd(nc, in_maps, core_ids=list(range(n_cores)))
    full = np.empty((Bn, 2, ROWS, N), np.float32)
    for c in range(n_cores):
        o = np.asarray(res.results[c]["out"]).reshape(docs_per_core, 2, ROWS, N)
        full[c * docs_per_core:(c + 1) * docs_per_core] = o.astype(np.float32)
    return full
